# revision 12
# baseline (speedup 1.0000x reference)
"""Trainium2 Bass kernel for nn_ClusterMemory (scatter_memory).

Computes:  loss = mean_b( logsumexp_n(20 * <x_b/|x_b|, f_n>) - 20*<x_b/|x_b|, f_{labels[indexes[b]]}> )

Strategy (8 NeuronCores, model/vocab parallel on the class axis N):
  - features [N=100000, 128] are transposed + cast to bf16 on the host, padded
    with zero rows to 102400 = 8 * 12800 and sharded column-wise: core c owns
    featT[:, c*12800:(c+1)*12800].  A zero row contributes exp(0)=1 to each
    row-sum; the host subtracts the pad count at the end.
  - normalized inputs (transposed, bf16, [128, 2048]) are replicated.
  - per core, a 3-stage pipeline over 112 chunks (16 b-blocks x 7 n-chunks):
      PE:  logits = xT_block.T @ featT_chunk  ->  PSUM ping/pong [128, 2048] f32
      ACT: exp(20 * logit)  PSUM -> SBUF bf16 ring (2 blocks deep)
      DVE: chained tensor_tensor_reduce row-sums  ->  Z[128, 16] f32
  - each core returns partial Z sums [128, 16] (b = bb*128 + p); the host
    all-reduces the 8 partials, takes log, and computes the picked-logit term
    (a 2048 x 128 dot) plus the final mean in float64.

logits are bounded by +-20 (both operands L2-normalized, temp=0.05), so the
unshifted exp is safe - no max-subtraction pass is needed.

The kernel is ACT-bound (exp runs at 1 elem/lane/cycle); everything else is
sized to stay off the critical path: hand-rolled semaphores (the HW-decoded
MM/ACT instructions only have one sync-wait slot), serialized input DMAs so
the first chunk lands early, and walrus LDWEIGHTS dedup re-enabled.
"""

import numpy as np
import ml_dtypes

B = 2048
D = 128
N = 100000
NCORES = 8
NLOC = 12800                      # per-core shard width (8*12800 = 102400)
NPAD = NCORES * NLOC - N          # 2400 zero rows, each contributing exp(0)=1
TEMP = 0.05
SCALE = 1.0 / TEMP
EPS = 1e-12
BBLOCKS = B // 128                # 16
# ACT processes PSUM in 4-bank [128, 2048] chunks (double-buffered in the 8
# PSUM banks); 12800 = 6*2048 + 512.
CHUNKS = [(i * 2048, 2048) for i in range(6)] + [(6 * 2048, 512)]
NCH = len(CHUNKS)

_NC = None          # cached Bass module
LAST_RESULTS = None  # BassKernelResults of the most recent run (for profiling)
_PATCHED = False
_WARMED = False


def _patch_ldw_opt():
    """Re-enable walrus LDWEIGHTS dedup (43us of redundant weight reloads
    otherwise: all 25 matmuls of a b-block share the same stationary xT
    block).  bass_utils hardcodes --enable-ldw-opt=false; rewrite the flag
    where the compiler command is spawned."""
    global _PATCHED
    if _PATCHED:
        return
    import concourse.bass_utils as bu

    orig = bu.run_command

    def patched(argv, **kwargs):
        argv = [
            "--enable-ldw-opt=true" if a == "--enable-ldw-opt=false" else a
            for a in argv
        ]
        return orig(argv, **kwargs)

    bu.run_command = patched
    _PATCHED = True


def _build_nc():
    import concourse.bass as bass
    from concourse import mybir

    NG = BBLOCKS * NCH  # 112 global chunks

    nc = bass.Bass(name="cluster_memory_lse")
    xT = nc.dram_tensor("xT", [D, B], mybir.dt.bfloat16, kind="ExternalInput")
    fT = nc.dram_tensor("fT", [D, NLOC], mybir.dt.bfloat16, kind="ExternalInput")
    zs = nc.dram_tensor("zs", [128, BBLOCKS], mybir.dt.float32, kind="ExternalOutput")

    with (
        nc.sbuf_tensor([D, B], mybir.dt.bfloat16) as xT_s,
        nc.sbuf_tensor([D, NLOC], mybir.dt.bfloat16) as fT_s,
        # exp output ring: 2 blocks x 7 chunks x 2048 (bf16)
        nc.sbuf_tensor([128, 2, NCH, 2048], mybir.dt.bfloat16) as ebuf,
        nc.sbuf_tensor([128, 2048], mybir.dt.bfloat16) as tout,   # ttr out scratch
        nc.sbuf_tensor([128, 512], mybir.dt.bfloat16) as zpad,    # zeros for tail pair
        nc.sbuf_tensor([128, 4], mybir.dt.float32) as partials,   # ttr accum chain
        nc.sbuf_tensor([128, BBLOCKS], mybir.dt.float32) as zs_s,
        nc.psum_tensor([128, 2048], mybir.dt.float32) as ps0,
        nc.psum_tensor([128, 2048], mybir.dt.float32) as ps1,
        nc.semaphore("dma_sem") as dma_sem,
        nc.semaphore("pe_sem") as pe_sem,
        nc.semaphore("act_sem") as act_sem,
        nc.semaphore("dve_sem") as dve_sem,
        nc.Block() as block,
    ):
        slots = [ps0, ps1]

        @block.sync
        def _(sync):
            # serialized input DMAs: the consumer walks chunks in order, and
            # concurrent queues would share HBM bandwidth and delay chunk 0
            # (13us to first matmul observed) - chained, the pipeline starts
            # after ~3us and the remaining loads hide under compute.
            sync.dma_start(out=xT_s[:, :], in_=xT[:, :]).then_inc(dma_sem, 16)
            for ci, (j0, w) in enumerate(CHUNKS):
                sync.wait_ge(dma_sem, 16 * (ci + 1))
                sync.dma_start(
                    out=fT_s[:, j0 : j0 + w], in_=fT[:, j0 : j0 + w]
                ).then_inc(dma_sem, 16)
            sync.wait_ge(dve_sem, BBLOCKS)
            sync.dma_start(out=zs[:, :], in_=zs_s[:, :]).then_inc(dma_sem, 16)
            sync.wait_ge(dma_sem, (NCH + 2) * 16)

        @block.tensor
        def _(tensor):
            for bb in range(BBLOCKS):
                w_ap = xT_s[:, bb * 128 : (bb + 1) * 128]
                for ci, (j0, w) in enumerate(CHUNKS):
                    g = bb * NCH + ci
                    ps = slots[g % 2]
                    if bb == 0:
                        # xT + fT chunks 0..ci loaded
                        tensor.wait_ge(dma_sem, 32 + 16 * ci)
                    for mi in range(w // 512):
                        inst = tensor.matmul(
                            ps[:, mi * 512 : (mi + 1) * 512],
                            lhsT=w_ap,
                            rhs=fT_s[:, j0 + mi * 512 : j0 + (mi + 1) * 512],
                            start=True,
                            stop=True,
                        )
                        if mi == 0 and g >= 2:
                            # slot release: ACT finished reading chunk g-2
                            # (transitively covers our own older writes)
                            inst._wait_ge(act_sem, g - 1)
                    inst.then_inc(pe_sem, 1)

        @block.scalar
        def _(scalar):
            # Dummy exp at stream start: pulls the ACT exp-table load into the
            # input-DMA window (first-execution table-load races were observed
            # to corrupt the first real activations otherwise).
            scalar.activation(
                out=partials[:, 0:1],
                in_=partials[:, 0:1],
                func=mybir.ActivationFunctionType.Exp,
                scale=0.0,
            )
            for bb in range(BBLOCKS):
                if bb >= 2:
                    # ring reuse: DVE consumed block bb-2
                    scalar.wait_ge(dve_sem, bb - 1)
                for ci, (j0, w) in enumerate(CHUNKS):
                    g = bb * NCH + ci
                    ps = slots[g % 2]
                    scalar.activation(
                        out=ebuf[:, bb % 2, ci, :w],
                        in_=ps[:, :w],
                        func=mybir.ActivationFunctionType.Exp,
                        scale=SCALE,
                    )._wait_ge(pe_sem, g + 1).then_inc(act_sem, 1)

        @block.vector
        def _(vector):
            vector.memset(zpad[:, :], 0.0)
            for bb in range(BBLOCKS):
                eb = ebuf[:, bb % 2]
                g0 = bb * NCH
                # fused pair-add + row-sum: partials[k] = sum(t_{2k} + t_{2k+1})
                for k in range(3):
                    vector.scalar_tensor_tensor(
                        out=tout[:, :],
                        in0=eb[:, 2 * k, :],
                        scalar=0.0,
                        in1=eb[:, 2 * k + 1, :],
                        op0=mybir.AluOpType.add,
                        op1=mybir.AluOpType.add,
                        accum_out=partials[:, k : k + 1],
                    )._wait_ge(act_sem, g0 + 2 * k + 2)
                # tail (512-wide chunk 6) paired with zeros
                vector.scalar_tensor_tensor(
                    out=tout[:, :512],
                    in0=eb[:, 6, :512],
                    scalar=0.0,
                    in1=zpad[:, :],
                    op0=mybir.AluOpType.add,
                    op1=mybir.AluOpType.add,
                    accum_out=partials[:, 3:4],
                )._wait_ge(act_sem, g0 + NCH)
                # Z column for this block = sum of the 4 partials
                vector.reduce_sum(
                    zs_s[:, bb : bb + 1], partials[:, :], axis=mybir.AxisListType.X
                ).then_inc(dve_sem, 1)

    return nc


def _get_nc():
    global _NC
    if _NC is None:
        _patch_ldw_opt()
        _NC = _build_nc()
    return _NC


def kernel(inputs, indexes, labels, features):
    global LAST_RESULTS
    from concourse.bass_utils import run_bass_kernel_spmd

    inputs = np.asarray(inputs, dtype=np.float32)
    features = np.asarray(features, dtype=np.float32)
    idx = np.asarray(indexes).astype(np.int64)
    lab = np.asarray(labels).astype(np.int64)

    # host prep: normalize inputs, transpose+cast both operands to bf16
    x64 = inputs.astype(np.float64)
    norms = np.maximum(np.sqrt((x64 * x64).sum(axis=1, keepdims=True)), EPS)
    xn = x64 / norms
    xT = np.ascontiguousarray(xn.T).astype(ml_dtypes.bfloat16)  # [128, 2048]

    fT_full = np.zeros((D, NCORES * NLOC), dtype=ml_dtypes.bfloat16)
    fT_full[:, :N] = features.T.astype(ml_dtypes.bfloat16)

    in_maps = [
        {
            "xT": xT,
            "fT": np.ascontiguousarray(fT_full[:, c * NLOC : (c + 1) * NLOC]),
        }
        for c in range(NCORES)
    ]

    nc = _get_nc()
    # Warm-up: the first execution after model load was observed to corrupt
    # block 0 on every core (ACT exp-table / DGE cold-start effects) - the
    # values come out plausible but ~5% off, so it cannot be detected from
    # the outputs.  Execute once and discard; subsequent runs are stable.
    global _WARMED
    if not _WARMED:
        run_bass_kernel_spmd(nc, in_maps, core_ids=list(range(NCORES)))
        _WARMED = True
    # Retry guard: a first-execution ACT-table-load race was observed to
    # corrupt one core's sums (inf) on a cold device.  Validate and re-run.
    for attempt in range(3):
        res = run_bass_kernel_spmd(nc, in_maps, core_ids=list(range(NCORES)))
        LAST_RESULTS = res
        Z = np.zeros((128, BBLOCKS), dtype=np.float64)
        for c in range(NCORES):
            Z += res.results[c]["zs"].astype(np.float64)
        # every row-sum must be finite and exceed its pad-only floor
        if np.isfinite(Z).all() and (Z > 0).all():
            break

    Zb = Z.T.reshape(-1)  # b = bb*128 + p
    Zb = Zb - float(NPAD)
    logz = np.log(Zb)

    targets = lab[idx]
    picked = SCALE * (xn * features[targets].astype(np.float64)).sum(axis=1)
    loss = (logz - picked).mean()
    return np.float32(loss)
